# revision 1
# baseline (speedup 1.0000x reference)
"""Trainium2 Bass kernel for vq-codebook CustomLinear.

Computes: out = x @ vector_bank[index].reshape(4096, 4096) + bias
  x:           [4, 2048, 4096] f32
  vector_bank: [2097152, 8] f32
  index:       [2097152] int32
  bias:        [4096] f32

Sharding (column-parallel over 8 NeuronCores): core c computes output
columns [c*512, (c+1)*512). Per core:
  - weight shard [4096, 512] gathered into SBUF via indirect DMA
    (HW consumes one offset per partition per instruction -> 2048 calls,
    the kernel's critical path on the GpSimd/SWDGE engine).
  - The GEMM is split into 4 K-windows of 8 k-tiles each; window s runs
    as soon as its k-tiles are gathered, spilling partial sums to DRAM
    scratch via HWDGE, so only ~1/4 of the PE work trails the gather.
  - x arrives row-major; each window re-reads its x column slice and
    PE-transposes it (contraction dim must sit on SBUF partitions).
  - Matmuls run in fp32r (full-rate fp32 PE mode, ~1.4e-4 rel err).
"""

import numpy as np

P = 128
B, S, K, NTOT = 4, 2048, 4096, 4096
M_ROWS = B * S  # 8192
NCORES = 8
NSHARD = NTOT // NCORES  # 512
VEC = 8
VPS = NSHARD // VEC  # 64 vectors per k-row per core
KT = K // P  # 32 k-tiles
MT = M_ROWS // P  # 64 m-tiles
NIDX = K * NTOT // VEC
SPLITS = [8, 8, 8, 8]  # k-tiles per K-window (sum = KT)
NSPLIT = len(SPLITS)

_CACHE = {}


def _build_nc():
    import concourse.bacc as bacc
    import concourse.bass as bass
    import concourse.mybir as mybir
    import concourse.tile as tile
    from concourse.masks import make_identity

    f32 = mybir.dt.float32
    f32r = mybir.dt.float32r
    i32 = mybir.dt.int32

    nc = bacc.Bacc("TRN2", target_bir_lowering=False, debug=False,
                   num_devices=NCORES)
    x = nc.dram_tensor("x", [M_ROWS, K], f32, kind="ExternalInput")
    bank = nc.dram_tensor("bank", [NIDX, VEC], f32, kind="ExternalInput")
    idx = nc.dram_tensor("idx", [K, VPS], i32, kind="ExternalInput")
    bias = nc.dram_tensor("bias", [P, NSHARD], f32, kind="ExternalInput")
    out = nc.dram_tensor("out", [M_ROWS, NSHARD], f32, kind="ExternalOutput")
    scratch = [nc.dram_tensor(f"scratch{s}", [M_ROWS, NSHARD], f32)
               for s in range(NSPLIT - 1)]

    GRP = 4  # max transposes per PSUM staging tile

    with tile.TileContext(nc) as tc:
        with (
            tc.tile_pool(name="wpool", bufs=1) as wpool,
            tc.tile_pool(name="xpool", bufs=3) as xpool,
            tc.tile_pool(name="xtpsum", bufs=2, space="PSUM") as xtpsum,
            tc.tile_pool(name="xtpool", bufs=4) as xtpool,
            tc.tile_pool(name="opsum", bufs=3, space="PSUM") as opsum,
            tc.tile_pool(name="opsum2", bufs=2, space="PSUM") as opsum2,
            tc.tile_pool(name="opool", bufs=3) as opool,
            tc.tile_pool(name="ppool", bufs=3) as ppool,
            tc.tile_pool(name="misc", bufs=1) as misc,
            tc.tile_pool(name="idxpool", bufs=1) as idxpool,
            tc.tile_pool(name="wgpool", bufs=4) as wgpool,
        ):
            ident = misc.tile([P, P], f32, name="ident")
            make_identity(nc, ident[:])
            bias_sb = misc.tile([P, NSHARD], f32, name="bias")
            nc.sync.dma_start(out=bias_sb[:], in_=bias[:])

            # Prefetch all index tiles so the gather stream never stalls.
            idx_tiles = []
            for k in range(KT):
                it = idxpool.tile([P, VPS], i32, name=f"idxt{k}")
                nc.sync.dma_start(out=it[:], in_=idx[k * P:(k + 1) * P, :])
                idx_tiles.append(it)

            # Gather the weight shard into SBUF: 32 resident f32r tiles
            # [128, 512]. fp32r matmul operands must be rounded by a compute
            # op, so gather into a scratch f32 ring and round-copy.
            w_tiles = []
            for k in range(KT):
                it = idx_tiles[k]
                wg = wgpool.tile([P, NSHARD], f32, name="wgather")
                for j in range(VPS):
                    nc.gpsimd.indirect_dma_start(
                        out=wg[:, j * VEC:(j + 1) * VEC],
                        out_offset=None,
                        in_=bank[:],
                        in_offset=bass.IndirectOffsetOnAxis(
                            ap=it[:, j:j + 1], axis=0),
                    )
                wt = wpool.tile([P, NSHARD], f32r, name=f"w{k}")
                if k % 2 == 0:
                    nc.vector.tensor_copy(out=wt[:], in_=wg[:])
                else:
                    nc.scalar.copy(out=wt[:], in_=wg[:])
                w_tiles.append(wt)

            # GEMM in K-windows; window s starts once its W k-tiles are
            # gathered. Partial sums spill to DRAM between windows. The last
            # window is small so little work trails the end of the gather.
            for s in range(NSPLIT):
                k0 = sum(SPLITS[:s])
                kt_s = SPLITS[s]
                kcols = kt_s * P
                for m in range(MT):
                    x_nat = xpool.tile([P, 8 * P], f32, name="xnat")
                    nc.sync.dma_start(
                        out=x_nat[:, :kcols],
                        in_=x[m * P:(m + 1) * P,
                              k0 * P:k0 * P + kcols])

                    xts = []
                    ngrp = (kt_s + GRP - 1) // GRP
                    for g in range(ngrp):
                        gw = min(GRP, kt_s - g * GRP)
                        xtp = xtpsum.tile([P, GRP * P], f32, name="xtp")
                        for j in range(gw):
                            kk = g * GRP + j
                            nc.tensor.transpose(
                                out=xtp[:, j * P:(j + 1) * P],
                                in_=x_nat[:, kk * P:(kk + 1) * P],
                                identity=ident[:],
                            )
                        xt = xtpool.tile([P, GRP * P], f32r, name="xt")
                        if (m + g) % 2 == 0:
                            nc.vector.tensor_copy(out=xt[:, :gw * P],
                                                  in_=xtp[:, :gw * P])
                        else:
                            nc.scalar.copy(out=xt[:, :gw * P],
                                           in_=xtp[:, :gw * P])
                        xts.append(xt)

                    last = s == NSPLIT - 1
                    # For the last window, keep the final k-tile (gathered
                    # last) out of the main PSUM group so the group closes —
                    # and its PSUM bank frees — before the gather finishes.
                    kt_main = kt_s - 1 if last else kt_s
                    ops = opsum.tile([P, NSHARD], f32, name="ops")
                    for kk in range(kt_main):
                        g, j = divmod(kk, GRP)
                        nc.tensor.matmul(
                            out=ops[:],
                            lhsT=xts[g][:, j * P:(j + 1) * P],
                            rhs=w_tiles[k0 + kk][:],
                            start=(kk == 0),
                            stop=(kk == kt_main - 1),
                        )

                    osb = opool.tile([P, NSHARD], f32, name="osb")
                    if s == 0:
                        nc.vector.tensor_add(out=osb[:], in0=ops[:],
                                             in1=bias_sb[:])
                    else:
                        prev = ppool.tile([P, NSHARD], f32, name="prev")
                        nc.sync.dma_start(
                            out=prev[:],
                            in_=scratch[s - 1][m * P:(m + 1) * P, :])
                        nc.vector.tensor_add(out=osb[:], in0=ops[:],
                                             in1=prev[:])
                    if last:
                        g, j = divmod(kt_s - 1, GRP)
                        ops2 = opsum2.tile([P, NSHARD], f32, name="ops2")
                        nc.tensor.matmul(
                            out=ops2[:],
                            lhsT=xts[g][:, j * P:(j + 1) * P],
                            rhs=w_tiles[k0 + kt_s - 1][:],
                            start=True, stop=True,
                        )
                        osb2 = opool.tile([P, NSHARD], f32, name="osb2")
                        nc.vector.tensor_add(out=osb2[:], in0=ops2[:],
                                             in1=osb[:])
                        osb = osb2
                    dst = out if last else scratch[s]
                    nc.sync.dma_start(out=dst[m * P:(m + 1) * P, :],
                                      in_=osb[:])

    nc.compile()
    return nc


def _get_nc():
    if "nc" not in _CACHE:
        _CACHE["nc"] = _build_nc()
    return _CACHE["nc"]


def kernel(x, vector_bank, index, bias):
    from concourse.bass_utils import run_bass_kernel_spmd

    x2 = np.ascontiguousarray(np.asarray(x, dtype=np.float32).reshape(M_ROWS, K))
    bank = np.ascontiguousarray(np.asarray(vector_bank, dtype=np.float32))
    idx3 = np.asarray(index, dtype=np.int32).reshape(K, NCORES, VPS)
    bias_f = np.asarray(bias, dtype=np.float32)

    in_maps = []
    for c in range(NCORES):
        in_maps.append({
            "x": x2,
            "bank": bank,
            "idx": np.ascontiguousarray(idx3[:, c, :]),
            "bias": np.ascontiguousarray(
                np.broadcast_to(bias_f[c * NSHARD:(c + 1) * NSHARD][None, :],
                                (P, NSHARD))),
        })

    nc = _get_nc()
    res = None
    last_err = None
    for _attempt in range(3):
        try:
            res = run_bass_kernel_spmd(nc, in_maps, list(range(NCORES)))
            break
        except Exception as e:  # transient device-unrecoverable on cold start
            last_err = e
    if res is None:
        raise last_err
    outs = [res.results[c]["out"] for c in range(NCORES)]
    full = np.concatenate(outs, axis=1).reshape(B, S, NTOT)
    return np.asarray(full, dtype=np.float32)

